# revision 21
# baseline (speedup 1.0000x reference)
"""Dempster-Shafer sequential fusion kernel for Trainium2 (Bass/Tile).

Reference computation (per batch row b):
    m = x[b, 0, :]
    for k in 1..D-1:
        alpha = x[b, k, :] + x[b, k, w]          # w = C-1 (omega channel)
        m     = m * alpha + m[w] * x[b, k, :]
        m     = m / sum(m)                        # renormalize each step
    out[b] = m

Key algebra: the combine is bilinear, so per-step normalization only changes
the per-row scale -> normalize once at the end.  Tracking s = m / m[w]
(note m[w] follows m[w] <- 3*m[w]*x[b,k,w] exactly) gives the scaled
recurrence
    s_k = (u_k * alpha_k) * s_{k-1} + u_k * x_k,   u_k = 1/(3 x[b,k,w])
    s_0 = x_0 / x_0[w]
which maps 1:1 onto the DVE tensor_tensor_scan op
    state = (data0[t] * state) + data1[t]

Layout: batch rows on partitions; per row the scan runs one chain per class
c over the D-1 steps.  The scan operands are built K-MAJOR (t rows x c cols,
row t=0 = per-class init state, rows 1..127 = steps) so both construct
passes are fully inner-contiguous; the scan itself walks them with a 3D
access pattern (c outer, t inner) — the hardware simply chains the
recurrence across dim boundaries in iteration order, and each chain's
leading reset position (data0=0) kills the incoming state, which was
verified exact on hardware.  C is processed in 3 slices for SBUF fit and
construct/scan pipelining.

Sharding: pure data parallel, batch axis split across 8 NeuronCores.
"""

import numpy as np

# Problem geometry (hardcoded per the harness contract).
B, D, C = 4096, 128, 101
N_CORES = 8
BC = B // N_CORES          # batch rows per core (512)
P = 128                    # SBUF partitions = rows per group
N_GROUPS = BC // P         # 4
D_SPLIT = 17               # steps 1..16 in scan A, 17..127 in scan B
T = D + 1                  # rows per class: 2 reset rows + D-1 steps
C_SLICES = ((0, 34), (34, 34), (68, 33))  # (start, width) over C

_CACHED = {}


def _scan_3d(nc, out, data0, data1, initial, op0, op1):
    """tensor_tensor_scan with multi-free-dim APs.

    Upstream bass asserts 2D operands, but the hardware streams the AP in
    iteration order and chains the recurrence across dim boundaries —
    exactly what the per-class reset positions need (verified exact on HW).
    """
    from concourse import mybir

    eng = nc.vector
    return eng.add_instruction(
        mybir.InstTensorScalarPtr(
            name=nc.get_next_instruction_name(),
            is_tensor_tensor_scan=True,
            is_scalar_tensor_tensor=True,
            op0=op0,
            op1=op1,
            ins=[
                eng.lower_ap(data0),
                eng.lower_ap_or_imm(initial),
                eng.lower_ap(data1),
            ],
            outs=[eng.lower_ap(out)],
        )
    )


def _build_nc():
    import contextlib

    import concourse.bacc as bacc
    import concourse.tile as tile
    from concourse import mybir

    f32 = mybir.dt.float32
    # Bacc (not plain Bass): its compile() runs generate_event_semaphores,
    # which splits multi-sem waits into EventSemaphore instructions — the
    # TRN2 ISA allows at most one sync wait per regular instruction.
    nc = bacc.Bacc("TRN2", target_bir_lowering=False, debug=True)
    x = nc.declare_dram_parameter("inputs", [BC, D, C], f32, isOutput=False)
    y = nc.declare_dram_parameter("output", [BC, C], f32, isOutput=True)

    # Load the source in two d-chunks so the next chunk/group's DMA overlaps
    # compute within the SBUF budget.  The first chunk is small so the very
    # first construct pass starts ~5us in instead of waiting ~17us for a
    # half-tile DMA.
    D_HALVES = ((1, 17), (17, 128))  # [start, end) over d

    with tile.TileContext(nc) as tc, contextlib.ExitStack() as ctx:
        xpool = ctx.enter_context(tc.tile_pool(name="xin", bufs=2))
        d0pool = ctx.enter_context(tc.tile_pool(name="d0", bufs=2))
        d1pool = ctx.enter_context(tc.tile_pool(name="d1", bufs=2))
        spool = ctx.enter_context(tc.tile_pool(name="small", bufs=3))

        for g in range(N_GROUPS):
            rows = slice(g * P, (g + 1) * P)

            # First source (init states) — issued before the bulk loads so it
            # doesn't queue behind megabytes in the FIFO HWDGE ring.
            x0 = spool.tile([P, C], f32, tag="x0")
            nc.sync.dma_start(out=x0, in_=x[rows, 0, :])
            # k-major source tiles: steps k=1..127 in two chunks.
            xts = []
            for hi, (da, db) in enumerate(D_HALVES):
                xt = xpool.tile([P, db - da, C], f32, tag=f"xt{hi}")
                nc.sync.dma_start(out=xt, in_=x[rows, da:db, :])
                xts.append(xt)

            # Per-step scalars: u_k = 1/(3 x_kw).
            u0 = spool.tile([P, 1], f32, tag="u0")
            nc.vector.reciprocal(u0, x0[:, C - 1 : C])
            # Per-chunk u so early constructs start as soon as their chunk's
            # DMA lands (a single full-width reciprocal would stall on the
            # largest chunk).
            uc = spool.tile([P, D - 1], f32, tag="uc")
            t3 = spool.tile([P, D - 1], f32, tag="t3")
            for hi, (da, db) in enumerate(D_HALVES):
                nc.vector.tensor_scalar_mul(
                    out=t3[:, da - 1 : db - 1],
                    in0=xts[hi][:, :, C - 1],
                    scalar1=3.0,
                )
                nc.vector.reciprocal(
                    uc[:, da - 1 : db - 1], t3[:, da - 1 : db - 1]
                )

            res = spool.tile([P, C], f32, tag="res")

            # Row layout per class: row 0 = reset A (init s0), rows 1..16 =
            # steps 1..16, row 17 = reset B (state hand-off), rows 18..128 =
            # steps 17..127.  Two scans per slice: scan A runs as soon as the
            # small first x-chunk lands, filling the big chunk's DMA latency.
            def row_of(k):
                return k if k < D_SPLIT else k + 1

            for c0, cw in C_SLICES:
                d0 = d0pool.tile([P, T, cw], f32, tag="d0")
                d1 = d1pool.tile([P, T, cw], f32, tag="d1")

                # Reset rows.
                nc.gpsimd.memset(d0[:, 0, :], 0.0)
                nc.gpsimd.memset(d0[:, D_SPLIT, :], 0.0)
                nc.vector.tensor_scalar_mul(
                    out=d1[:, 0, :], in0=x0[:, c0 : c0 + cw], scalar1=u0
                )

                # Step rows: all-contiguous construct passes, per d-chunk.
                for hi, (da, db) in enumerate(D_HALVES):
                    ra, rb = row_of(da), row_of(db - 1) + 1
                    u_b = (
                        uc[:, da - 1 : db - 1]
                        .unsqueeze(2)
                        .to_broadcast([P, db - da, cw])
                    )
                    nc.vector.tensor_mul(
                        out=d1[:, ra:rb, :],
                        in0=xts[hi][:, :, c0 : c0 + cw],
                        in1=u_b,
                    )
                    # alpha' = y + y_w, and y_w = u*x_w == 1/3 exactly.
                    nc.vector.tensor_scalar_add(
                        out=d0[:, ra:rb, :],
                        in0=d1[:, ra:rb, :],
                        scalar1=1.0 / 3.0,
                    )

                # Scans with 3D APs: c outer, t inner; per-class chains,
                # chained across classes through the reset rows.  In-place
                # (out == data1): each position's write happens pipeline-
                # stages after its read, so overwriting d1 is safe and saves
                # a third large tile.  After scan A, d1 row D_SPLIT-1 holds
                # the per-class state at step 16; hand it to scan B's reset.
                ota = spool.tile([P, D_SPLIT, cw], f32, tag="ota")
                _scan_3d(
                    nc,
                    out=ota.transpose([0, 2, 1]),
                    data0=d0[:, :D_SPLIT, :].transpose([0, 2, 1]),
                    data1=d1[:, :D_SPLIT, :].transpose([0, 2, 1]),
                    initial=0.0,
                    op0=mybir.AluOpType.mult,
                    op1=mybir.AluOpType.add,
                )
                nc.vector.tensor_copy(d1[:, D_SPLIT, :], ota[:, D_SPLIT - 1, :])
                _scan_3d(
                    nc,
                    out=d1[:, D_SPLIT:, :].transpose([0, 2, 1]),
                    data0=d0[:, D_SPLIT:, :].transpose([0, 2, 1]),
                    data1=d1[:, D_SPLIT:, :].transpose([0, 2, 1]),
                    initial=0.0,
                    op0=mybir.AluOpType.mult,
                    op1=mybir.AluOpType.add,
                )

                # Final states for this slice.
                nc.scalar.copy(res[:, c0 : c0 + cw], d1[:, T - 1, :])

            # Normalize -> store.
            ssum = spool.tile([P, 1], f32, tag="ssum")
            nc.vector.reduce_sum(ssum, res, axis=mybir.AxisListType.X)
            rec = spool.tile([P, 1], f32, tag="rec")
            nc.vector.reciprocal(rec, ssum)
            res2 = spool.tile([P, C], f32, tag="res2")
            nc.scalar.mul(out=res2, in_=res, mul=rec)
            nc.sync.dma_start(out=y[rows, :], in_=res2)

    nc.compile()
    return nc


def _get_nc():
    if "nc" not in _CACHED:
        _CACHED["nc"] = _build_nc()
    return _CACHED["nc"]


def kernel(inputs: np.ndarray) -> np.ndarray:
    from concourse.bass_utils import run_bass_kernel_spmd

    inputs = np.asarray(inputs, dtype=np.float32)
    assert inputs.shape == (B, D, C), inputs.shape

    nc = _get_nc()
    in_maps = [
        {"inputs": np.ascontiguousarray(inputs[i * BC : (i + 1) * BC])}
        for i in range(N_CORES)
    ]
    out = run_bass_kernel_spmd(nc, in_maps, list(range(N_CORES)))
    return np.concatenate(
        [out.results[i]["output"] for i in range(N_CORES)], axis=0
    )


# revision 24
# speedup vs baseline: 1.0366x; 1.0366x over previous
"""Dempster-Shafer sequential fusion kernel for Trainium2 (Bass/Tile).

Reference computation (per batch row b):
    m = x[b, 0, :]
    for k in 1..D-1:
        alpha = x[b, k, :] + x[b, k, w]          # w = C-1 (omega channel)
        m     = m * alpha + m[w] * x[b, k, :]
        m     = m / sum(m)                        # renormalize each step
    out[b] = m

Key algebra: the combine is bilinear, so per-step normalization only changes
the per-row scale -> normalize once at the end.  Tracking s = m / m[w]
(note m[w] follows m[w] <- 3*m[w]*x[b,k,w] exactly) gives the scaled
recurrence
    s_k = (u_k * alpha_k) * s_{k-1} + u_k * x_k,   u_k = 1/(3 x[b,k,w])
    s_0 = x_0 / x_0[w]
which maps 1:1 onto the DVE tensor_tensor_scan op
    state = (data0[t] * state) + data1[t]

Layout: batch rows on partitions; per row the scan runs one chain per class
c over the D-1 steps.  The scan operands are built K-MAJOR (t rows x c cols,
row t=0 = per-class init state, rows 1..127 = steps) so both construct
passes are fully inner-contiguous; the scan itself walks them with a 3D
access pattern (c outer, t inner) — the hardware simply chains the
recurrence across dim boundaries in iteration order, and each chain's
leading reset position (data0=0) kills the incoming state, which was
verified exact on hardware.  C is processed in 3 slices for SBUF fit and
construct/scan pipelining.

Sharding: pure data parallel, batch axis split across 8 NeuronCores.
"""

import numpy as np

# Problem geometry (hardcoded per the harness contract).
B, D, C = 4096, 128, 101
N_CORES = 8
BC = B // N_CORES          # batch rows per core (512)
P = 128                    # SBUF partitions = rows per group
N_GROUPS = BC // P         # 4
D_SPLIT = 41               # group 0: steps 1..40 in scan A, 41..127 in scan B
C_SLICES = ((0, 34), (34, 34), (68, 33))  # (start, width) over C

_CACHED = {}


def _scan_3d(nc, out, data0, data1, initial, op0, op1):
    """tensor_tensor_scan with multi-free-dim APs.

    Upstream bass asserts 2D operands, but the hardware streams the AP in
    iteration order and chains the recurrence across dim boundaries —
    exactly what the per-class reset positions need (verified exact on HW).
    """
    from concourse import mybir

    eng = nc.vector
    return eng.add_instruction(
        mybir.InstTensorScalarPtr(
            name=nc.get_next_instruction_name(),
            is_tensor_tensor_scan=True,
            is_scalar_tensor_tensor=True,
            op0=op0,
            op1=op1,
            ins=[
                eng.lower_ap(data0),
                eng.lower_ap_or_imm(initial),
                eng.lower_ap(data1),
            ],
            outs=[eng.lower_ap(out)],
        )
    )


def _build_nc():
    import contextlib

    import concourse.bacc as bacc
    import concourse.tile as tile
    from concourse import mybir

    f32 = mybir.dt.float32
    # Bacc (not plain Bass): its compile() runs generate_event_semaphores,
    # which splits multi-sem waits into EventSemaphore instructions — the
    # TRN2 ISA allows at most one sync wait per regular instruction.
    nc = bacc.Bacc("TRN2", target_bir_lowering=False, debug=True)
    x = nc.declare_dram_parameter("inputs", [BC, D, C], f32, isOutput=False)
    y = nc.declare_dram_parameter("output", [BC, C], f32, isOutput=True)

    # Load the source in two d-chunks so the next chunk/group's DMA overlaps
    # compute within the SBUF budget.  The split matches group 0's scan
    # split: its chunk-A constructs + scan A fill chunk B's DMA latency.
    D_HALVES = ((1, D_SPLIT), (D_SPLIT, 128))  # [start, end) over d

    with tile.TileContext(nc) as tc, contextlib.ExitStack() as ctx:
        xpool = ctx.enter_context(tc.tile_pool(name="xin", bufs=2))
        d0pool = ctx.enter_context(tc.tile_pool(name="d0", bufs=2))
        d1pool = ctx.enter_context(tc.tile_pool(name="d1", bufs=2))
        spool = ctx.enter_context(tc.tile_pool(name="small", bufs=3))

        for g in range(N_GROUPS):
            rows = slice(g * P, (g + 1) * P)

            # First source (init states) — issued before the bulk loads so it
            # doesn't queue behind megabytes in the FIFO HWDGE ring.
            x0 = spool.tile([P, C], f32, tag="x0")
            nc.sync.dma_start(out=x0, in_=x[rows, 0, :])
            # k-major source tiles: steps k=1..127 in two chunks.
            xts = []
            for hi, (da, db) in enumerate(D_HALVES):
                xt = xpool.tile([P, db - da, C], f32, tag=f"xt{hi}")
                nc.sync.dma_start(out=xt, in_=x[rows, da:db, :])
                xts.append(xt)

            # Per-step scalars: u_k = 1/(3 x_kw).
            u0 = spool.tile([P, 1], f32, tag="u0")
            nc.vector.reciprocal(u0, x0[:, C - 1 : C])
            # Per-chunk u so early constructs start as soon as their chunk's
            # DMA lands (a single full-width reciprocal would stall on the
            # largest chunk).
            uc = spool.tile([P, D - 1], f32, tag="uc")
            t3 = spool.tile([P, D - 1], f32, tag="t3")
            for hi, (da, db) in enumerate(D_HALVES):
                nc.vector.tensor_scalar_mul(
                    out=t3[:, da - 1 : db - 1],
                    in0=xts[hi][:, :, C - 1],
                    scalar1=3.0,
                )
                nc.vector.reciprocal(
                    uc[:, da - 1 : db - 1], t3[:, da - 1 : db - 1]
                )

            res = spool.tile([P, C], f32, tag="res")

            # Group 0 runs at kernel start with nothing to hide its chunk-B
            # DMA latency, so its scan is split in two: row 0 = reset A
            # (init s0), rows 1..D_SPLIT-1 = steps 1..40, row D_SPLIT =
            # reset B (state hand-off), then steps 41..127.  Later groups'
            # DMAs prefetch under earlier groups' scans: single scan.
            split = g == 0
            tg = D + 1 if split else D

            def row_of(k):
                return k + 1 if (split and k >= D_SPLIT) else k

            for c0, cw in C_SLICES:
                d0 = d0pool.tile([P, tg, cw], f32, tag="d0")
                d1 = d1pool.tile([P, tg, cw], f32, tag="d1")

                # Reset rows.
                nc.gpsimd.memset(d0[:, 0, :], 0.0)
                if split:
                    nc.gpsimd.memset(d0[:, D_SPLIT, :], 0.0)
                nc.vector.tensor_scalar_mul(
                    out=d1[:, 0, :], in0=x0[:, c0 : c0 + cw], scalar1=u0
                )

                # Step rows: all-contiguous construct passes, per d-chunk.
                for hi, (da, db) in enumerate(D_HALVES):
                    ra, rb = row_of(da), row_of(db - 1) + 1
                    u_b = (
                        uc[:, da - 1 : db - 1]
                        .unsqueeze(2)
                        .to_broadcast([P, db - da, cw])
                    )
                    nc.vector.tensor_mul(
                        out=d1[:, ra:rb, :],
                        in0=xts[hi][:, :, c0 : c0 + cw],
                        in1=u_b,
                    )
                    # alpha' = y + y_w, and y_w = u*x_w == 1/3 exactly.
                    nc.vector.tensor_scalar_add(
                        out=d0[:, ra:rb, :],
                        in0=d1[:, ra:rb, :],
                        scalar1=1.0 / 3.0,
                    )

                # Scan(s) with 3D APs: c outer, t inner; per-class chains,
                # chained across classes through the reset rows.  In-place
                # (out == data1): each position's write happens pipeline-
                # stages after its read, so overwriting d1 is safe and saves
                # a third large tile.
                def scan_rows(ra, rb, out_tile=None):
                    _scan_3d(
                        nc,
                        out=(out_tile if out_tile is not None else d1[:, ra:rb, :]).transpose(
                            [0, 2, 1]
                        ),
                        data0=d0[:, ra:rb, :].transpose([0, 2, 1]),
                        data1=d1[:, ra:rb, :].transpose([0, 2, 1]),
                        initial=0.0,
                        op0=mybir.AluOpType.mult,
                        op1=mybir.AluOpType.add,
                    )

                if split:
                    ota = spool.tile([P, D_SPLIT, cw], f32, tag="ota")
                    scan_rows(0, D_SPLIT, out_tile=ota)
                    nc.vector.tensor_copy(
                        d1[:, D_SPLIT, :], ota[:, D_SPLIT - 1, :]
                    )
                    scan_rows(D_SPLIT, tg)
                else:
                    scan_rows(0, tg)

                # Final states for this slice.
                nc.scalar.copy(res[:, c0 : c0 + cw], d1[:, tg - 1, :])

            # Normalize -> store.
            ssum = spool.tile([P, 1], f32, tag="ssum")
            nc.vector.reduce_sum(ssum, res, axis=mybir.AxisListType.X)
            rec = spool.tile([P, 1], f32, tag="rec")
            nc.vector.reciprocal(rec, ssum)
            res2 = spool.tile([P, C], f32, tag="res2")
            nc.scalar.mul(out=res2, in_=res, mul=rec)
            nc.sync.dma_start(out=y[rows, :], in_=res2)

    nc.compile()
    return nc


def _get_nc():
    if "nc" not in _CACHED:
        _CACHED["nc"] = _build_nc()
    return _CACHED["nc"]


def kernel(inputs: np.ndarray) -> np.ndarray:
    from concourse.bass_utils import run_bass_kernel_spmd

    inputs = np.asarray(inputs, dtype=np.float32)
    assert inputs.shape == (B, D, C), inputs.shape

    nc = _get_nc()
    in_maps = [
        {"inputs": np.ascontiguousarray(inputs[i * BC : (i + 1) * BC])}
        for i in range(N_CORES)
    ]
    out = run_bass_kernel_spmd(nc, in_maps, list(range(N_CORES)))
    return np.concatenate(
        [out.results[i]["output"] for i in range(N_CORES)], axis=0
    )
